# revision 63
# baseline (speedup 1.0000x reference)
"""Multi-head causal attention (B=2, S=2048, H=16, DH=64, D=1024) on 8 TRN2 cores.

Sharding: Megatron tensor-parallel over heads - core c owns heads {2c, 2c+1}:
  * column-slices of Wq/Wk/Wv (128 cols each) + bias slices,
  * row-slice of Wo (128 rows),
  * full hidden_states (pre-transposed on host to [D, B*S]).
Each core computes a partial output (its 2 heads through Wo rows); host sums
the 8 f16 partials (row-parallel unshard) and adds bo.

Device dataflow per core, interleaved so PE (matmul), ACT (exp) and DVE
(evacuations) overlap across phases; 512-token groups alternate batches so
attention blocks unlock progressively:
  per group g (b, j):
  A) QKV: qT/kT/vT [128, 512] = W_slice.T @ hiddenT chunk (contraction over D
     in 8 chunks of 128, f32 PSUM accumulate; bias added on evacuation).
     vT columns transposed into V_aug [tok128, chunk, head, 128] via PE
     transpose; V_aug cols 64:128 are pre-set to ones so AV matmuls emit the
     softmax denominator replicated on PSUM partitions 64:127.
  B) one 512-wide attention macro-block per group, both heads, causal:
     one wave per kv chunk of 128: scoresT[kv,q] = kT.T @ qT (K=64, N=512,
     heads on PE row-groups 0-63/64-127), exp via ACT, diagonal chunks
     stream only their valid q range and get a triangular GPSIMD mask,
     then ct[128, q] += V_aug.T @ expT (rows 0:64 ctx, 64:128 denom).
     N=512 AV matmuls fully hide the 128-row LDWEIGHTS that N=256 paid
     ~48ns each for. Normalize: DVE recip of rows 64:128 -> mul into ctxT.
  C) output projection for the finished 512 tokens, deferred into the next
     step's waves and interleaved ~1:2 with QKV quanta (each O-proj matmul
     WAR-waits on the DVE evac of the chunk two back, psA bufs=2); the
     final group splits normalize+proj into col pieces [0:256]/[256:512]
     so most of it overlaps the last waves and ships early.
Prologue: weights bundled into 2 DMAs (wqk on sync, wrest=wv+wo+tri+id on
sync) + split xg0 on scalar - each ring pays a serial ~2-3us completion
receipt per DMA, so fewer/bundled prologue DMAs start compute ~7us sooner.
"""
import os
import sys

sys.path.insert(0, "/opt/trn_rl_repo")

from contextlib import ExitStack

import numpy as np

import concourse.bass as bass
import concourse.mybir as mybir
import concourse.tile as tile
from concourse import bacc
from concourse.bass_utils import run_bass_kernel_spmd

F32 = mybir.dt.float32
F16 = mybir.dt.float16
MM_DT = F16
MM_NP = np.float16

B, S, H, DH = 2, 2048, 16, 64
D = H * DH            # 1024
T = B * S             # 4096 tokens
NCORES = 8
HPC = H // NCORES     # 2 heads per core
KC = D // 128         # 8 contraction chunks
NG = T // 512         # 8 token groups
GPB = NG // B         # 4 groups per batch
NKV = T // 128        # 32 kv chunks of 128 tokens
EXPFN = mybir.ActivationFunctionType.Exp

# DMA-xbar transpose serializes the whole HWDGE ring under Tile's
# hang-workaround (measured ~1.2us each + completion waits) - PE default.
USE_DMAT = os.environ.get("KERNEL_DMAT") == "1"
SP_DT = F32  # matmul outputs must be fp32 in PSUM


def _body(nc, tc, ctx, t_in, t_out, t_out_dbg=None):
    xt, wq, wk, wv, wrest, bqkv = t_in
    po = t_out

    const = ctx.enter_context(tc.tile_pool(name="const", bufs=1))
    big = ctx.enter_context(tc.tile_pool(name="big", bufs=1))
    xtp = ctx.enter_context(tc.tile_pool(name="xtp", bufs=4))
    ep = ctx.enter_context(tc.tile_pool(name="ep", bufs=8))
    rp = ctx.enter_context(tc.tile_pool(name="rp", bufs=4))
    # 8 bufs = one staging tile per group, zero reuse: a reused ostb's WAR
    # wait rides an aliased DMA-lane counter that other po stores advance,
    # which released the next writer early once O-proj chunks were
    # deferred multiple steps (nondeterministic corruption)
    osp = ctx.enter_context(tc.tile_pool(name="osp", bufs=8))
    vsp = ctx.enter_context(tc.tile_pool(name="vsp", bufs=3))

    # PSUM budget (8 banks):
    #   sp f32 2 banks x2 + ct0/ct1 1 bank x1 each + accop 1 bank x2 = 8
    ct_bufs = 1
    psS = ctx.enter_context(tc.tile_pool(name="psS", bufs=2, space="PSUM"))
    psC = ctx.enter_context(tc.tile_pool(name="psC", bufs=ct_bufs, space="PSUM"))
    psA = ctx.enter_context(tc.tile_pool(name="psA", bufs=2, space="PSUM"))

    # ---- constants / weights in SBUF (bundled: one DMA per ring pays one
    # serial completion receipt, vs ~2-3us each for 7 separate prologue DMAs)
    wq_s = const.tile([128, KC, 128], MM_DT, tag="wq")
    wk_s = const.tile([128, KC, 128], MM_DT, tag="wk")
    wv_s = const.tile([128, KC, 128], MM_DT, tag="wv")
    wrest_s = const.tile([128, D + 256], MM_DT, tag="wrest")
    wo_s = wrest_s[:, 0:D]
    tri_s = wrest_s[:, D:D + 128]
    id_s = wrest_s[:, D + 128:D + 256]
    bqkv_s = const.tile([128, 3], F32, tag="bqkv")
    bq_s, bk_s, bv_s = (bqkv_s[:, 0:1], bqkv_s[:, 1:2], bqkv_s[:, 2:3])
    qT = big.tile([128, T], MM_DT, tag="qT")
    kT = big.tile([128, T], MM_DT, tag="kT")
    vT = big.tile([128, T], MM_DT, tag="vT")
    ctxT = big.tile([128, T], MM_DT, tag="ctxT")
    vaug = big.tile([128, NKV, HPC, 128], MM_DT, tag="vaug")

    woc = wo_s.rearrange("p (c n) -> p c n", c=KC)

    if t_out_dbg is not None:
        dbgp = ctx.enter_context(tc.tile_pool(name="dbgp", bufs=1))
        dbg_craw_s = dbgp.tile([128, T], F32, tag="craw")
        dbg_den_s = dbgp.tile([128, T], F32, tag="den")

    # groups alternate batches so attention work unlocks early and evenly
    steps = [(b, j) for j in range(GPB) for b in range(B)]

    xgs = {}

    def emit_xg_load(idx, engine=None, split=False):
        # prefetch the hidden-state slice for step idx (gpsimd/SWDGE ring so
        # it is not queued behind the po stores on the sync ring); split=True
        # halves the transfer so the first chain can start on the first half
        gg_l = steps[idx][0] * GPB + steps[idx][1]
        xg = xtp.tile([128, KC, 512], MM_DT, tag="xt")
        eng = engine or nc.gpsimd
        if split:
            # 6/2 split: the q chain consumes chunks in order at ~2 chunks
            # per 432ns, so the first piece must cover it until the second
            # piece's ring receipt clears
            eng.dma_start(xg[:, 0:6, :], xt[gg_l][:, 0:6, :])
            eng.dma_start(xg[:, 6:, :], xt[gg_l][:, 6:, :])
        else:
            eng.dma_start(xg[:], xt[gg_l])
        xgs[idx] = xg

    def qkv_quanta(idx):
        """Per-step QKV work broken into small PE quanta so it can be
        interleaved into the previous step's attention waves."""
        (b_q, j_q) = steps[idx]
        gg_q = b_q * GPB + j_q
        cols_q = slice(gg_q * 512, (gg_q + 1) * 512)
        thunks = []
        state = {}
        for (w_s, b_s, dst) in ((wq_s, bq_s, qT), (wk_s, bk_s, kT),
                                (wv_s, bv_s, vT)):
            def chain(k0, w_s=w_s, b_s=b_s, dst=dst):
                if k0 == 0:
                    state['acc'] = psA.tile([128, 512], F32, tag="accop",
                                            name="acc")
                acc = state['acc']
                for k in (k0, k0 + 1):
                    nc.tensor.matmul(acc[:], w_s[:, k, :], xgs[idx][:, k, :],
                                     start=(k == 0), stop=(k == KC - 1))
                if k0 == KC - 2:
                    nc.vector.tensor_scalar_add(dst[:, cols_q], acc[:],
                                                b_s[:])
            for k0 in range(0, KC, 2):
                thunks.append(lambda k0=k0, chain=chain: chain(k0))
        for i in range(4):
            def trans(i=i):
                cg = gg_q * 4 + i
                vslice = vT[:, cg * 128:(cg + 1) * 128]
                tp = psA.tile([128, 128], MM_DT, tag="accop")
                nc.tensor.transpose(tp[:], vslice, id_s[:])
                nc.vector.tensor_copy(
                    vaug[:, cg, :, 0:64],
                    tp[:].rearrange("p (h d) -> p h d", h=HPC))
            thunks.append(trans)
        return thunks

    # PE warmup: dummy matmuls on a memset tile so the HAM clock-gate opens
    # (~3.4us of activity) before the first real QKV chain arrives - no
    # data deps, so these start right after the kernel preamble
    warm = const.tile([128, 512], MM_DT, tag="warm")
    nc.vector.memset(warm[:], 0.0)
    # 12 matmuls ~= 5us at mid-clock: opens the HAM clock-gate AND bridges
    # the idle window until wq + xg0 land (~1.6us past an 8-matmul warmup)
    wacc = psA.tile([128, 512], F32, tag="accop", name="wacc")
    for k in range(16):
        nc.tensor.matmul(wacc[:], warm[:, 0:128], warm[:],
                         start=(k == 0), stop=(k == 15))

    # prologue: each ring pays a serial ~2-3us completion receipt per DMA.
    # need-by times: wq ~11 (q chain), wk ~12.9 (k chain), bqkv ~14 (the q
    # evac frees the accop buffer the v chain allocates), wv ~14.4,
    # tri/id ~16-19. bqkv is tiny so it leads sync without delaying wq
    # much; wv leads the gpsimd/SWDGE ring whose first item lands ~14us.
    nc.sync.dma_start(bqkv_s[:], bqkv[:])
    nc.sync.dma_start(wq_s[:], wq[:])
    nc.sync.dma_start(wk_s[:], wk[:])
    emit_xg_load(0, nc.scalar, split=True)
    nc.gpsimd.dma_start(wv_s[:], wv[:])
    nc.gpsimd.dma_start(wrest_s[:], wrest[:])
    emit_xg_load(1, nc.sync)
    # ones columns 64:128 of V_aug (softmax denominator, replicated over the
    # 64 PSUM partitions 64:127 so normalize needs no partition broadcast).
    # memset, not DMA: a DMA's completion lands on a shared DMA-lane counter
    # that later po writes keep advancing, so every AV matmul would pick up
    # false waits on po-write completions.
    va_ones = vaug[:].rearrange("p c h x -> p (c h) x")[:, :, 64:128]
    nc.gpsimd.memset(va_ones, 1.0)
    emit_xg_load(2)
    for t in qkv_quanta(0):
        t()
    pending_oproj = []

    for idx, (b, j) in enumerate(steps):
        gg = b * GPB + j
        cols = slice(gg * 512, (gg + 1) * 512)
        n_waves = 4 * j + 4
        last_step = (idx == len(steps) - 1)

        # work to interleave into this step's attention waves: deferred
        # output projections, next step's QKV, and an xg prefetch
        from collections import deque as _dq
        quanta = _dq()
        if idx + 3 < len(steps):
            quanta.append(lambda i=idx + 3: emit_xg_load(i))
        # interleave deferred O-proj chunks between QKV thunks: each O-proj
        # matmul WAR-waits on the evac (DVE, ~690ns) of the chunk two back
        # (psA bufs=2), so consecutive O-proj quanta must not be adjacent
        # or the PE stalls on the evac. Take at most one O-proj chunk per
        # wave: early (few-wave) steps would otherwise cluster them, and
        # capping here shifts the backlog into the late attention-heavy
        # steps whose waves run dry of QKV fill.
        nxt = qkv_quanta(idx + 1) if idx + 1 < len(steps) else []
        # from mid-kernel on, hold back ~8 chunks so the final step (which
        # has no next-QKV fill) still has PE work to cover its exposed
        # normalize chains
        reserve = 8 if idx >= 4 else 0
        take = (len(pending_oproj) if last_step else
                min(len(pending_oproj), n_waves,
                    max(0, len(pending_oproj) - reserve)))
        a, bq_ = pending_oproj[:take], list(nxt)
        pending_oproj = pending_oproj[take:]
        ratio = max(1, -(-len(bq_) // max(1, len(a))))
        while a or bq_:
            for _ in range(ratio):
                if bq_:
                    quanta.append(bq_.pop(0))
            if a:
                quanta.append(a.pop(0))
        waves_left = [2 * n_waves]

        def pop_quanta():
            # even draw across remaining half-waves (one draw after exp,
            # one after av)
            k = -(-len(quanta) // max(1, waves_left[0]))
            for _ in range(k):
                if quanta:
                    quanta.popleft()()
            waves_left[0] -= 1

        # ---- attention macro-block (b, j): q cols [gg*512,(gg+1)*512).
        # One wave per kv chunk; N=512 matmuls fully hide the 128-row
        # LDWEIGHTS that N=256 AV matmuls paid ~48ns each for, and the
        # diagonal chunks only stream their valid q range.
        qc0 = b * S + j * 512
        qcols = slice(qc0, qc0 + 512)
        nch = 4 * j + 4                # kv chunks of 128 for this block
        ct0 = psC.tile([128, 512], F32, tag="ct0")
        ct1 = psC.tile([128, 512], F32, tag="ct1")
        cts = [ct0, ct1]

        def av_wave(jj, e, qlo):
            first = (jj == 0)
            last = (jj == nch - 1)
            for h in range(HPC):
                lhsT = vaug[:, b * (S // 128) + jj, h, :]
                nc.tensor.matmul(cts[h][:, qlo:512], lhsT,
                                 e[:, h, qlo:512], start=first, stop=last)

        if last_step:
            # final group: normalize + O-proj split into two column pieces.
            # cols [0:256] of ct are final right after av(nch-3) (the d=1
            # diagonal chunk), so that piece overlaps the last waves; only
            # the [256:512] chain remains exposed after the final av.
            ostF = osp.tile([128, KC, 512], MM_DT, tag="ost", name="ostF")

            def final_piece(c0, c1, gg=gg):
                w = c1 - c0
                for h in range(HPC):
                    dn = rp.tile([64, w], F32, tag=f"dF{c0}")
                    nc.vector.tensor_copy(dn[:], cts[h][64:128, c0:c1])
                    rr = rp.tile([64, w], F32, tag=f"rF{c0}")
                    nc.vector.reciprocal_approx_fast(rr[:], dn[:])
                    nc.vector.tensor_mul(
                        ctxT[h * 64:(h + 1) * 64, qc0 + c0:qc0 + c1],
                        cts[h][0:64, c0:c1], rr[:])
                for c in range(KC):
                    op = psA.tile([128, w], SP_DT, tag="accop", name="opf")
                    nc.tensor.matmul(op[:], woc[:, c, :],
                                     ctxT[:, qc0 + c0:qc0 + c1],
                                     start=True, stop=True)
                    nc.vector.tensor_copy(ostF[:, c, c0:c1], op[:])
                    # ship each piece as soon as its 4-chunk group is done:
                    # piece 1 (cols 0:256) leaves during the last waves, so
                    # only ~512KB of piece 2 remains for the drain to wait on
                    if c % 4 == 3:
                        eng = nc.sync if c == 3 else nc.scalar
                        eng.dma_start(po[gg][:, c - 3:c + 1, c0:c1],
                                      ostF[:, c - 3:c + 1, c0:c1])

        # software pipeline: emit scores(jj)+exp(jj) before AV(jj-1), so
        # the PE stream is S0 S1 A0 S2 A1 ... and exp overlaps AV
        prev = None
        for jj in range(nch):
            d = jj - 4 * j             # diagonal index (>=0 on diagonal)
            qlo = max(0, d * 128)      # q cols below qlo are fully masked
            kcols = slice(b * S + jj * 128, b * S + jj * 128 + 128)
            sp = psS.tile([128, HPC, 512], SP_DT, tag="sp")
            for h in range(HPC):
                nc.tensor.matmul(
                    sp[:, h, qlo:512],
                    kT[h * 64:(h + 1) * 64, kcols],
                    qT[h * 64:(h + 1) * 64, qc0 + qlo:qc0 + 512],
                    start=True, stop=True)
            e = ep.tile([128, HPC, 512], MM_DT, tag="e")
            nc.scalar.activation(e[:, :, qlo:512], sp[:, :, qlo:512],
                                 EXPFN, scale=0.125)
            if d >= 0:                 # mask the diagonal 128x128 triangle
                for h in range(HPC):
                    nc.gpsimd.tensor_mul(e[:, h, qlo:qlo + 128],
                                         e[:, h, qlo:qlo + 128], tri_s)
            pop_quanta()
            if prev is not None:
                av_wave(*prev)
                if last_step and prev[0] == nch - 3:
                    final_piece(0, 256)
            pop_quanta()
            prev = (jj, e, qlo)
        av_wave(*prev)
        if last_step:
            final_piece(256, 512)
        else:
            # normalize: ctxT[:, q] = ct[0:64] * recip(ct[64:128]) - the
            # denominator is already replicated across partitions 64:127
            for h in range(HPC):
                # tensor_copy is the only DVE op that honors a partition
                # shift; custom-DVE (recip) and TT reads do not, so copy the
                # replicated denominators down to partitions 0:63 first
                dn = rp.tile([64, 512], F32, tag="d")
                nc.vector.tensor_copy(dn[:], cts[h][64:128, :])
                r = rp.tile([64, 512], F32, tag="r")
                nc.vector.reciprocal_approx_fast(r[:], dn[:])
                nc.vector.tensor_mul(ctxT[h * 64:(h + 1) * 64, qcols],
                                     cts[h][0:64, :], r[:])
                if t_out_dbg is not None:
                    nc.vector.tensor_copy(
                        dbg_craw_s[h * 64:(h + 1) * 64, qcols],
                        cts[h][0:64, :])
                    nc.vector.tensor_copy(
                        dbg_den_s[h * 64:(h + 1) * 64, qcols],
                        cts[h][64:128, :])

        while quanta:          # leftover next-step QKV work
            quanta.popleft()()

        if not last_step:
            # ---- output projection for the completed 512-token group,
            # deferred into the next step's waves (the last step's is
            # handled by final_piece above).
            ostb = osp.tile([128, KC, 512], MM_DT, tag="ost", name="ostb")

            def proj_chunk(c, cols=cols, ostb=ostb, gg=gg):
                op = psA.tile([128, 512], SP_DT, tag="accop", name="op")
                nc.tensor.matmul(op[:], woc[:, c, :], ctxT[:, cols],
                                 start=True, stop=True)
                nc.vector.tensor_copy(ostb[:, c, :], op[:])
                if c == KC - 1:
                    nc.sync.dma_start(po[gg], ostb[:])

            # bind proj_chunk now: it is redefined each step, and these
            # thunks may run 2+ steps later
            pending_oproj.extend(
                lambda c=c, f=proj_chunk: f(c) for c in range(KC))

    if t_out_dbg is not None:
        dq, dk, dc, dv, dcr, dde = t_out_dbg
        st = ctx.enter_context(tc.tile_pool(name="dbg", bufs=1))
        for src, dst in ((qT, dq), (kT, dk), (ctxT, dc)):
            tmp = st.tile([128, T], F32, tag="dbgt")
            nc.vector.tensor_copy(tmp[:], src[:])
            nc.sync.dma_start(dst[:], tmp[:])
        nc.sync.dma_start(dcr[:], dbg_craw_s[:])
        nc.sync.dma_start(dde[:], dbg_den_s[:])
        tmpv = st.tile([128, NKV * HPC * 128], F32, tag="dbgt")
        nc.vector.tensor_copy(
            tmpv[:], vaug[:].rearrange("p c h x -> p (c h x)"))
        nc.sync.dma_start(dv[:], tmpv[:])


_NC = None


def _build():
    global _NC
    if _NC is not None:
        return _NC
    nc = bacc.Bacc("TRN2", target_bir_lowering=False, debug=False,
                   num_devices=NCORES)
    t_in = [
        nc.dram_tensor("xt", [NG, 128, KC, 512], MM_DT, kind="ExternalInput").ap(),
        nc.dram_tensor("wq", [128, KC, 128], MM_DT, kind="ExternalInput").ap(),
        nc.dram_tensor("wk", [128, KC, 128], MM_DT, kind="ExternalInput").ap(),
        nc.dram_tensor("wv", [128, KC, 128], MM_DT, kind="ExternalInput").ap(),
        nc.dram_tensor("wrest", [128, D + 256], MM_DT,
                       kind="ExternalInput").ap(),
        nc.dram_tensor("bqkv", [128, 3], F32, kind="ExternalInput").ap(),
    ]
    # group-major, partition-major layout: each group store is one DMA with
    # 8KB contiguous per partition (vs 1KB segments in a [KC,128,T] layout)
    po = nc.dram_tensor("po", [NG, 128, KC, 512], MM_DT,
                        kind="ExternalOutput").ap()
    t_out_dbg = None
    if os.environ.get("KERNEL_DEBUG_TAPS") == "1":
        t_out_dbg = [
            nc.dram_tensor("dbg_qT", [128, T], F32, kind="ExternalOutput").ap(),
            nc.dram_tensor("dbg_kT", [128, T], F32, kind="ExternalOutput").ap(),
            nc.dram_tensor("dbg_ctxT", [128, T], F32, kind="ExternalOutput").ap(),
            nc.dram_tensor("dbg_vaug", [128, NKV * HPC * 128], F32,
                           kind="ExternalOutput").ap(),
            nc.dram_tensor("dbg_craw", [128, T], F32, kind="ExternalOutput").ap(),
            nc.dram_tensor("dbg_den", [128, T], F32, kind="ExternalOutput").ap(),
        ]
    with tile.TileContext(nc) as tc, ExitStack() as ctx:
        _body(nc, tc, ctx, t_in, po, t_out_dbg)
    nc.compile()
    _NC = nc
    return nc


def _in_maps(hidden_states, Wq, bq, Wk, bk, Wv, bv, Wo, bo):
    hid = np.asarray(hidden_states, dtype=np.float32).reshape(T, D)
    hidT = hid.T.astype(MM_NP)                       # [D, T]
    xt = np.ascontiguousarray(
        hidT.reshape(KC, 128, NG, 512).transpose(2, 1, 0, 3))
    tri = np.triu(np.ones((128, 128), MM_NP))
    eye = np.eye(128, dtype=MM_NP)
    common = {"xt": xt}
    maps = []
    for c in range(NCORES):
        cs = slice(c * 128, (c + 1) * 128)
        wslc = lambda W: np.asarray(W)[:, cs].astype(MM_NP).reshape(
            KC, 128, 128).transpose(1, 0, 2)
        wo_c = np.asarray(Wo)[cs, :].astype(MM_NP)
        maps.append(dict(
            common,
            wq=np.ascontiguousarray(wslc(Wq)),
            wk=np.ascontiguousarray(wslc(Wk)),
            wv=np.ascontiguousarray(wslc(Wv)),
            wrest=np.ascontiguousarray(np.concatenate(
                [wo_c, tri, eye], axis=1)),
            bqkv=np.ascontiguousarray(np.stack(
                [np.asarray(bq)[cs], np.asarray(bk)[cs],
                 np.asarray(bv)[cs]], axis=1).astype(np.float32)),
        ))
    return maps


def kernel(hidden_states, Wq, bq, Wk, bk, Wv, bv, Wo, bo):
    nc = _build()
    maps = _in_maps(hidden_states, Wq, bq, Wk, bk, Wv, bv, Wo, bo)
    res = run_bass_kernel_spmd(nc, maps, list(range(NCORES))).results
    acc = np.zeros((NG, 128, KC, 512), np.float32)
    for r in res:
        acc += r["po"].astype(np.float32)
    # [NG,128,KC,512] -> [KC*128, NG*512] = [D, T]
    outT = acc.transpose(2, 1, 0, 3).reshape(D, T)
    out = outT.T + np.asarray(bo, dtype=np.float32)[None, :]
    return out.reshape(B, S, D).astype(np.float32)



# revision 64
# speedup vs baseline: 1.1800x; 1.1800x over previous
"""Multi-head causal attention (B=2, S=2048, H=16, DH=64, D=1024) on 8 TRN2 cores.

Sharding: Megatron tensor-parallel over heads - core c owns heads {2c, 2c+1}:
  * column-slices of Wq/Wk/Wv (128 cols each) + bias slices,
  * row-slice of Wo (128 rows),
  * full hidden_states (pre-transposed on host to [D, B*S]).
Each core computes a partial output (its 2 heads through Wo rows); host sums
the 8 f16 partials (row-parallel unshard) and adds bo.

Device dataflow per core, interleaved so PE (matmul), ACT (exp) and DVE
(evacuations) overlap across phases; 512-token groups alternate batches so
attention blocks unlock progressively:
  per group g (b, j):
  A) QKV: qT/kT/vT [128, 512] = W_slice.T @ hiddenT chunk (contraction over D
     in 8 chunks of 128, f32 PSUM accumulate; bias added on evacuation).
     vT columns transposed into V_aug [tok128, chunk, head, 128] via PE
     transpose; V_aug cols 64:128 are pre-set to ones so AV matmuls emit the
     softmax denominator replicated on PSUM partitions 64:127.
  B) one 512-wide attention macro-block per group, both heads, causal:
     one wave per kv chunk of 128: scoresT[kv,q] = kT.T @ qT (K=64, N=512,
     heads on PE row-groups 0-63/64-127), exp via ACT, diagonal chunks
     stream only their valid q range and get a triangular GPSIMD mask,
     then ct[128, q] += V_aug.T @ expT (rows 0:64 ctx, 64:128 denom).
     N=512 AV matmuls fully hide the 128-row LDWEIGHTS that N=256 paid
     ~48ns each for. Normalize: DVE recip of rows 64:128 -> mul into ctxT.
  C) output projection for the finished 512 tokens, deferred into the next
     step's waves and interleaved ~1:2 with QKV quanta (each O-proj matmul
     WAR-waits on the DVE evac of the chunk two back, psA bufs=2); the
     final group splits normalize+proj into col pieces [0:256]/[256:512]
     so most of it overlaps the last waves and ships early.
Prologue: weights bundled into 2 DMAs (wqk on sync, wrest=wv+wo+tri+id on
sync) + split xg0 on scalar - each ring pays a serial ~2-3us completion
receipt per DMA, so fewer/bundled prologue DMAs start compute ~7us sooner.
"""
import os
import sys

sys.path.insert(0, "/opt/trn_rl_repo")

from contextlib import ExitStack

import numpy as np

import concourse.bass as bass
import concourse.mybir as mybir
import concourse.tile as tile
from concourse import bacc
from concourse.bass_utils import run_bass_kernel_spmd

F32 = mybir.dt.float32
F16 = mybir.dt.float16
MM_DT = F16
MM_NP = np.float16

B, S, H, DH = 2, 2048, 16, 64
D = H * DH            # 1024
T = B * S             # 4096 tokens
NCORES = 8
HPC = H // NCORES     # 2 heads per core
KC = D // 128         # 8 contraction chunks
NG = T // 512         # 8 token groups
GPB = NG // B         # 4 groups per batch
NKV = T // 128        # 32 kv chunks of 128 tokens
EXPFN = mybir.ActivationFunctionType.Exp

# DMA-xbar transpose serializes the whole HWDGE ring under Tile's
# hang-workaround (measured ~1.2us each + completion waits) - PE default.
USE_DMAT = os.environ.get("KERNEL_DMAT") == "1"
SP_DT = F32  # matmul outputs must be fp32 in PSUM


def _body(nc, tc, ctx, t_in, t_out, t_out_dbg=None):
    xt, wq, wk, wv, wrest, bqkv = t_in
    po = t_out

    const = ctx.enter_context(tc.tile_pool(name="const", bufs=1))
    big = ctx.enter_context(tc.tile_pool(name="big", bufs=1))
    xtp = ctx.enter_context(tc.tile_pool(name="xtp", bufs=4))
    ep = ctx.enter_context(tc.tile_pool(name="ep", bufs=8))
    rp = ctx.enter_context(tc.tile_pool(name="rp", bufs=4))
    # 8 bufs = one staging tile per group, zero reuse: a reused ostb's WAR
    # wait rides an aliased DMA-lane counter that other po stores advance,
    # which released the next writer early once O-proj chunks were
    # deferred multiple steps (nondeterministic corruption)
    osp = ctx.enter_context(tc.tile_pool(name="osp", bufs=8))
    vsp = ctx.enter_context(tc.tile_pool(name="vsp", bufs=3))

    # PSUM budget (8 banks):
    #   sp f32 2 banks x2 + ct0/ct1 1 bank x1 each + accop 1 bank x2 = 8
    ct_bufs = 1
    psS = ctx.enter_context(tc.tile_pool(name="psS", bufs=2, space="PSUM"))
    psC = ctx.enter_context(tc.tile_pool(name="psC", bufs=ct_bufs, space="PSUM"))
    psA = ctx.enter_context(tc.tile_pool(name="psA", bufs=2, space="PSUM"))

    # ---- constants / weights in SBUF (bundled: one DMA per ring pays one
    # serial completion receipt, vs ~2-3us each for 7 separate prologue DMAs)
    wq_s = const.tile([128, KC, 128], MM_DT, tag="wq")
    wk_s = const.tile([128, KC, 128], MM_DT, tag="wk")
    wv_s = const.tile([128, KC, 128], MM_DT, tag="wv")
    wrest_s = const.tile([128, D + 256], MM_DT, tag="wrest")
    wo_s = wrest_s[:, 0:D]
    tri_s = wrest_s[:, D:D + 128]
    id_s = wrest_s[:, D + 128:D + 256]
    bqkv_s = const.tile([128, 3], F32, tag="bqkv")
    bq_s, bk_s, bv_s = (bqkv_s[:, 0:1], bqkv_s[:, 1:2], bqkv_s[:, 2:3])
    qT = big.tile([128, T], MM_DT, tag="qT")
    kT = big.tile([128, T], MM_DT, tag="kT")
    vT = big.tile([128, T], MM_DT, tag="vT")
    ctxT = big.tile([128, T], MM_DT, tag="ctxT")
    vaug = big.tile([128, NKV, HPC, 128], MM_DT, tag="vaug")

    woc = wo_s.rearrange("p (c n) -> p c n", c=KC)

    if t_out_dbg is not None:
        dbgp = ctx.enter_context(tc.tile_pool(name="dbgp", bufs=1))
        dbg_craw_s = dbgp.tile([128, T], F32, tag="craw")
        dbg_den_s = dbgp.tile([128, T], F32, tag="den")

    # groups alternate batches so attention work unlocks early and evenly
    steps = [(b, j) for j in range(GPB) for b in range(B)]

    xgs = {}

    def emit_xg_load(idx, engine=None, split=False):
        # prefetch the hidden-state slice for step idx (gpsimd/SWDGE ring so
        # it is not queued behind the po stores on the sync ring); split=True
        # halves the transfer so the first chain can start on the first half
        gg_l = steps[idx][0] * GPB + steps[idx][1]
        xg = xtp.tile([128, KC, 512], MM_DT, tag="xt")
        eng = engine or nc.gpsimd
        if split:
            # 6/2 split: the q chain consumes chunks in order at ~2 chunks
            # per 432ns, so the first piece must cover it until the second
            # piece's ring receipt clears
            eng.dma_start(xg[:, 0:6, :], xt[gg_l][:, 0:6, :])
            eng.dma_start(xg[:, 6:, :], xt[gg_l][:, 6:, :])
        else:
            eng.dma_start(xg[:], xt[gg_l])
        xgs[idx] = xg

    def qkv_quanta(idx):
        """Per-step QKV work broken into small PE quanta so it can be
        interleaved into the previous step's attention waves."""
        (b_q, j_q) = steps[idx]
        gg_q = b_q * GPB + j_q
        cols_q = slice(gg_q * 512, (gg_q + 1) * 512)
        thunks = []
        state = {}
        for (w_s, b_s, dst) in ((wq_s, bq_s, qT), (wk_s, bk_s, kT),
                                (wv_s, bv_s, vT)):
            def chain(k0, w_s=w_s, b_s=b_s, dst=dst):
                if k0 == 0:
                    state['acc'] = psA.tile([128, 512], F32, tag="accop",
                                            name="acc")
                acc = state['acc']
                for k in (k0, k0 + 1):
                    nc.tensor.matmul(acc[:], w_s[:, k, :], xgs[idx][:, k, :],
                                     start=(k == 0), stop=(k == KC - 1))
                if k0 == KC - 2:
                    nc.vector.tensor_scalar_add(dst[:, cols_q], acc[:],
                                                b_s[:])
            for k0 in range(0, KC, 2):
                thunks.append(lambda k0=k0, chain=chain: chain(k0))
        for i in range(4):
            def trans(i=i):
                cg = gg_q * 4 + i
                vslice = vT[:, cg * 128:(cg + 1) * 128]
                tp = psA.tile([128, 128], MM_DT, tag="accop")
                nc.tensor.transpose(tp[:], vslice, id_s[:])
                nc.vector.tensor_copy(
                    vaug[:, cg, :, 0:64],
                    tp[:].rearrange("p (h d) -> p h d", h=HPC))
            thunks.append(trans)
        return thunks

    # PE warmup: dummy matmuls on a memset tile so the HAM clock-gate opens
    # (~3.4us of activity) before the first real QKV chain arrives - no
    # data deps, so these start right after the kernel preamble
    warm = const.tile([128, 512], MM_DT, tag="warm")
    nc.vector.memset(warm[:], 0.0)
    # 12 matmuls ~= 5us at mid-clock: opens the HAM clock-gate AND bridges
    # the idle window until wq + xg0 land (~1.6us past an 8-matmul warmup)
    wacc = psA.tile([128, 512], F32, tag="accop", name="wacc")
    for k in range(12):
        nc.tensor.matmul(wacc[:], warm[:, 0:128], warm[:],
                         start=(k == 0), stop=(k == 11))

    # prologue: each ring pays a serial ~2-3us completion receipt per DMA.
    # need-by times: wq ~11 (q chain), wk ~12.9 (k chain), bqkv ~14 (the q
    # evac frees the accop buffer the v chain allocates), wv ~14.4,
    # tri/id ~16-19. bqkv is tiny so it leads sync without delaying wq
    # much; wv leads the gpsimd/SWDGE ring whose first item lands ~14us.
    nc.sync.dma_start(bqkv_s[:], bqkv[:])
    nc.sync.dma_start(wq_s[:], wq[:])
    nc.sync.dma_start(wk_s[:], wk[:])
    emit_xg_load(0, nc.scalar, split=True)
    nc.gpsimd.dma_start(wv_s[:], wv[:])
    nc.gpsimd.dma_start(wrest_s[:], wrest[:])
    emit_xg_load(1, nc.sync)
    # ones columns 64:128 of V_aug (softmax denominator, replicated over the
    # 64 PSUM partitions 64:127 so normalize needs no partition broadcast).
    # memset, not DMA: a DMA's completion lands on a shared DMA-lane counter
    # that later po writes keep advancing, so every AV matmul would pick up
    # false waits on po-write completions.
    va_ones = vaug[:].rearrange("p c h x -> p (c h) x")[:, :, 64:128]
    nc.gpsimd.memset(va_ones, 1.0)
    emit_xg_load(2)
    for t in qkv_quanta(0):
        t()
    pending_oproj = []

    for idx, (b, j) in enumerate(steps):
        gg = b * GPB + j
        cols = slice(gg * 512, (gg + 1) * 512)
        n_waves = 4 * j + 4
        last_step = (idx == len(steps) - 1)

        # work to interleave into this step's attention waves: deferred
        # output projections, next step's QKV, and an xg prefetch
        from collections import deque as _dq
        quanta = _dq()
        if idx + 3 < len(steps):
            quanta.append(lambda i=idx + 3: emit_xg_load(i))
        # interleave deferred O-proj chunks between QKV thunks: each O-proj
        # matmul WAR-waits on the evac (DVE, ~690ns) of the chunk two back
        # (psA bufs=2), so consecutive O-proj quanta must not be adjacent
        # or the PE stalls on the evac. Take at most one O-proj chunk per
        # wave: early (few-wave) steps would otherwise cluster them, and
        # capping here shifts the backlog into the late attention-heavy
        # steps whose waves run dry of QKV fill.
        nxt = qkv_quanta(idx + 1) if idx + 1 < len(steps) else []
        # from mid-kernel on, hold back ~8 chunks so the final step (which
        # has no next-QKV fill) still has PE work to cover its exposed
        # normalize chains
        reserve = 8 if idx >= 4 else 0
        take = (len(pending_oproj) if last_step else
                min(len(pending_oproj), n_waves,
                    max(0, len(pending_oproj) - reserve)))
        a, bq_ = pending_oproj[:take], list(nxt)
        pending_oproj = pending_oproj[take:]
        ratio = max(1, -(-len(bq_) // max(1, len(a))))
        while a or bq_:
            for _ in range(ratio):
                if bq_:
                    quanta.append(bq_.pop(0))
            if a:
                quanta.append(a.pop(0))
        waves_left = [2 * n_waves]

        def pop_quanta():
            # even draw across remaining half-waves (one draw after exp,
            # one after av)
            k = -(-len(quanta) // max(1, waves_left[0]))
            for _ in range(k):
                if quanta:
                    quanta.popleft()()
            waves_left[0] -= 1

        # ---- attention macro-block (b, j): q cols [gg*512,(gg+1)*512).
        # One wave per kv chunk; N=512 matmuls fully hide the 128-row
        # LDWEIGHTS that N=256 AV matmuls paid ~48ns each for, and the
        # diagonal chunks only stream their valid q range.
        qc0 = b * S + j * 512
        qcols = slice(qc0, qc0 + 512)
        nch = 4 * j + 4                # kv chunks of 128 for this block
        ct0 = psC.tile([128, 512], F32, tag="ct0")
        ct1 = psC.tile([128, 512], F32, tag="ct1")
        cts = [ct0, ct1]

        def av_wave(jj, e, qlo):
            first = (jj == 0)
            last = (jj == nch - 1)
            for h in range(HPC):
                lhsT = vaug[:, b * (S // 128) + jj, h, :]
                nc.tensor.matmul(cts[h][:, qlo:512], lhsT,
                                 e[:, h, qlo:512], start=first, stop=last)

        if last_step:
            # final group: normalize + O-proj split into two column pieces.
            # cols [0:256] of ct are final right after av(nch-3) (the d=1
            # diagonal chunk), so that piece overlaps the last waves; only
            # the [256:512] chain remains exposed after the final av.
            ostF = osp.tile([128, KC, 512], MM_DT, tag="ost", name="ostF")

            def final_piece(c0, c1, gg=gg):
                w = c1 - c0
                for h in range(HPC):
                    dn = rp.tile([64, w], F32, tag=f"dF{c0}")
                    nc.vector.tensor_copy(dn[:], cts[h][64:128, c0:c1])
                    rr = rp.tile([64, w], F32, tag=f"rF{c0}")
                    nc.vector.reciprocal_approx_fast(rr[:], dn[:])
                    nc.vector.tensor_mul(
                        ctxT[h * 64:(h + 1) * 64, qc0 + c0:qc0 + c1],
                        cts[h][0:64, c0:c1], rr[:])
                for c in range(KC):
                    op = psA.tile([128, w], SP_DT, tag="accop", name="opf")
                    nc.tensor.matmul(op[:], woc[:, c, :],
                                     ctxT[:, qc0 + c0:qc0 + c1],
                                     start=True, stop=True)
                    nc.vector.tensor_copy(ostF[:, c, c0:c1], op[:])
                    # ship each piece as soon as its 4-chunk group is done:
                    # piece 1 (cols 0:256) leaves during the last waves, so
                    # only ~512KB of piece 2 remains for the drain to wait on
                    if c % 4 == 3:
                        eng = nc.sync if c == 3 else nc.scalar
                        eng.dma_start(po[gg][:, c - 3:c + 1, c0:c1],
                                      ostF[:, c - 3:c + 1, c0:c1])

        # software pipeline: emit scores(jj)+exp(jj) before AV(jj-1), so
        # the PE stream is S0 S1 A0 S2 A1 ... and exp overlaps AV
        prev = None
        for jj in range(nch):
            d = jj - 4 * j             # diagonal index (>=0 on diagonal)
            qlo = max(0, d * 128)      # q cols below qlo are fully masked
            kcols = slice(b * S + jj * 128, b * S + jj * 128 + 128)
            sp = psS.tile([128, HPC, 512], SP_DT, tag="sp")
            for h in range(HPC):
                nc.tensor.matmul(
                    sp[:, h, qlo:512],
                    kT[h * 64:(h + 1) * 64, kcols],
                    qT[h * 64:(h + 1) * 64, qc0 + qlo:qc0 + 512],
                    start=True, stop=True)
            e = ep.tile([128, HPC, 512], MM_DT, tag="e")
            nc.scalar.activation(e[:, :, qlo:512], sp[:, :, qlo:512],
                                 EXPFN, scale=0.125)
            if d >= 0:                 # mask the diagonal 128x128 triangle
                for h in range(HPC):
                    nc.gpsimd.tensor_mul(e[:, h, qlo:qlo + 128],
                                         e[:, h, qlo:qlo + 128], tri_s)
            pop_quanta()
            if prev is not None:
                av_wave(*prev)
                if last_step and prev[0] == nch - 3:
                    final_piece(0, 256)
            pop_quanta()
            prev = (jj, e, qlo)
        av_wave(*prev)
        if last_step:
            final_piece(256, 512)
        else:
            # normalize: ctxT[:, q] = ct[0:64] * recip(ct[64:128]) - the
            # denominator is already replicated across partitions 64:127
            for h in range(HPC):
                # tensor_copy is the only DVE op that honors a partition
                # shift; custom-DVE (recip) and TT reads do not, so copy the
                # replicated denominators down to partitions 0:63 first
                dn = rp.tile([64, 512], F32, tag="d")
                nc.vector.tensor_copy(dn[:], cts[h][64:128, :])
                r = rp.tile([64, 512], F32, tag="r")
                nc.vector.reciprocal_approx_fast(r[:], dn[:])
                nc.vector.tensor_mul(ctxT[h * 64:(h + 1) * 64, qcols],
                                     cts[h][0:64, :], r[:])
                if t_out_dbg is not None:
                    nc.vector.tensor_copy(
                        dbg_craw_s[h * 64:(h + 1) * 64, qcols],
                        cts[h][0:64, :])
                    nc.vector.tensor_copy(
                        dbg_den_s[h * 64:(h + 1) * 64, qcols],
                        cts[h][64:128, :])

        while quanta:          # leftover next-step QKV work
            quanta.popleft()()

        if not last_step:
            # ---- output projection for the completed 512-token group,
            # deferred into the next step's waves (the last step's is
            # handled by final_piece above).
            ostb = osp.tile([128, KC, 512], MM_DT, tag="ost", name="ostb")

            def proj_chunk(c, cols=cols, ostb=ostb, gg=gg):
                op = psA.tile([128, 512], SP_DT, tag="accop", name="op")
                nc.tensor.matmul(op[:], woc[:, c, :], ctxT[:, cols],
                                 start=True, stop=True)
                nc.vector.tensor_copy(ostb[:, c, :], op[:])
                if c == KC - 1:
                    nc.sync.dma_start(po[gg], ostb[:])

            # bind proj_chunk now: it is redefined each step, and these
            # thunks may run 2+ steps later
            pending_oproj.extend(
                lambda c=c, f=proj_chunk: f(c) for c in range(KC))

    if t_out_dbg is not None:
        dq, dk, dc, dv, dcr, dde = t_out_dbg
        st = ctx.enter_context(tc.tile_pool(name="dbg", bufs=1))
        for src, dst in ((qT, dq), (kT, dk), (ctxT, dc)):
            tmp = st.tile([128, T], F32, tag="dbgt")
            nc.vector.tensor_copy(tmp[:], src[:])
            nc.sync.dma_start(dst[:], tmp[:])
        nc.sync.dma_start(dcr[:], dbg_craw_s[:])
        nc.sync.dma_start(dde[:], dbg_den_s[:])
        tmpv = st.tile([128, NKV * HPC * 128], F32, tag="dbgt")
        nc.vector.tensor_copy(
            tmpv[:], vaug[:].rearrange("p c h x -> p (c h x)"))
        nc.sync.dma_start(dv[:], tmpv[:])


_NC = None


def _build():
    global _NC
    if _NC is not None:
        return _NC
    nc = bacc.Bacc("TRN2", target_bir_lowering=False, debug=False,
                   num_devices=NCORES)
    t_in = [
        nc.dram_tensor("xt", [NG, 128, KC, 512], MM_DT, kind="ExternalInput").ap(),
        nc.dram_tensor("wq", [128, KC, 128], MM_DT, kind="ExternalInput").ap(),
        nc.dram_tensor("wk", [128, KC, 128], MM_DT, kind="ExternalInput").ap(),
        nc.dram_tensor("wv", [128, KC, 128], MM_DT, kind="ExternalInput").ap(),
        nc.dram_tensor("wrest", [128, D + 256], MM_DT,
                       kind="ExternalInput").ap(),
        nc.dram_tensor("bqkv", [128, 3], F32, kind="ExternalInput").ap(),
    ]
    # group-major, partition-major layout: each group store is one DMA with
    # 8KB contiguous per partition (vs 1KB segments in a [KC,128,T] layout)
    po = nc.dram_tensor("po", [NG, 128, KC, 512], MM_DT,
                        kind="ExternalOutput").ap()
    t_out_dbg = None
    if os.environ.get("KERNEL_DEBUG_TAPS") == "1":
        t_out_dbg = [
            nc.dram_tensor("dbg_qT", [128, T], F32, kind="ExternalOutput").ap(),
            nc.dram_tensor("dbg_kT", [128, T], F32, kind="ExternalOutput").ap(),
            nc.dram_tensor("dbg_ctxT", [128, T], F32, kind="ExternalOutput").ap(),
            nc.dram_tensor("dbg_vaug", [128, NKV * HPC * 128], F32,
                           kind="ExternalOutput").ap(),
            nc.dram_tensor("dbg_craw", [128, T], F32, kind="ExternalOutput").ap(),
            nc.dram_tensor("dbg_den", [128, T], F32, kind="ExternalOutput").ap(),
        ]
    with tile.TileContext(nc) as tc, ExitStack() as ctx:
        _body(nc, tc, ctx, t_in, po, t_out_dbg)
    nc.compile()
    _NC = nc
    return nc


def _in_maps(hidden_states, Wq, bq, Wk, bk, Wv, bv, Wo, bo):
    hid = np.asarray(hidden_states, dtype=np.float32).reshape(T, D)
    hidT = hid.T.astype(MM_NP)                       # [D, T]
    xt = np.ascontiguousarray(
        hidT.reshape(KC, 128, NG, 512).transpose(2, 1, 0, 3))
    tri = np.triu(np.ones((128, 128), MM_NP))
    eye = np.eye(128, dtype=MM_NP)
    common = {"xt": xt}
    maps = []
    for c in range(NCORES):
        cs = slice(c * 128, (c + 1) * 128)
        wslc = lambda W: np.asarray(W)[:, cs].astype(MM_NP).reshape(
            KC, 128, 128).transpose(1, 0, 2)
        wo_c = np.asarray(Wo)[cs, :].astype(MM_NP)
        maps.append(dict(
            common,
            wq=np.ascontiguousarray(wslc(Wq)),
            wk=np.ascontiguousarray(wslc(Wk)),
            wv=np.ascontiguousarray(wslc(Wv)),
            wrest=np.ascontiguousarray(np.concatenate(
                [wo_c, tri, eye], axis=1)),
            bqkv=np.ascontiguousarray(np.stack(
                [np.asarray(bq)[cs], np.asarray(bk)[cs],
                 np.asarray(bv)[cs]], axis=1).astype(np.float32)),
        ))
    return maps


def kernel(hidden_states, Wq, bq, Wk, bk, Wv, bv, Wo, bo):
    nc = _build()
    maps = _in_maps(hidden_states, Wq, bq, Wk, bk, Wv, bv, Wo, bo)
    res = run_bass_kernel_spmd(nc, maps, list(range(NCORES))).results
    acc = np.zeros((NG, 128, KC, 512), np.float32)
    for r in res:
        acc += r["po"].astype(np.float32)
    # [NG,128,KC,512] -> [KC*128, NG*512] = [D, T]
    outT = acc.transpose(2, 1, 0, 3).reshape(D, T)
    out = outT.T + np.asarray(bo, dtype=np.float32)[None, :]
    return out.reshape(B, S, D).astype(np.float32)



# revision 66
# speedup vs baseline: 1.1953x; 1.0130x over previous
"""Multi-head causal attention (B=2, S=2048, H=16, DH=64, D=1024) on 8 TRN2 cores.

Sharding: Megatron tensor-parallel over heads - core c owns heads {2c, 2c+1}:
  * column-slices of Wq/Wk/Wv (128 cols each) + bias slices,
  * row-slice of Wo (128 rows),
  * full hidden_states (pre-transposed on host to [D, B*S]).
Each core computes a partial output (its 2 heads through Wo rows); host sums
the 8 f16 partials (row-parallel unshard) and adds bo.

Device dataflow per core, interleaved so PE (matmul), ACT (exp) and DVE
(evacuations) overlap across phases; 512-token groups alternate batches so
attention blocks unlock progressively:
  per group g (b, j):
  A) QKV: qT/kT/vT [128, 512] = W_slice.T @ hiddenT chunk (contraction over D
     in 8 chunks of 128, f32 PSUM accumulate; bias added on evacuation).
     vT columns transposed into V_aug [tok128, chunk, head, 128] via PE
     transpose; V_aug cols 64:128 are pre-set to ones so AV matmuls emit the
     softmax denominator replicated on PSUM partitions 64:127.
  B) one 512-wide attention macro-block per group, both heads, causal:
     one wave per kv chunk of 128: scoresT[kv,q] = kT.T @ qT (K=64, N=512,
     heads on PE row-groups 0-63/64-127), exp via ACT, diagonal chunks
     stream only their valid q range and get a triangular GPSIMD mask,
     then ct[128, q] += V_aug.T @ expT (rows 0:64 ctx, 64:128 denom).
     N=512 AV matmuls fully hide the 128-row LDWEIGHTS that N=256 paid
     ~48ns each for. Normalize: DVE recip of rows 64:128 -> mul into ctxT.
  C) output projection for the finished 512 tokens, deferred into the next
     step's waves and interleaved ~1:2 with QKV quanta (each O-proj matmul
     WAR-waits on the DVE evac of the chunk two back, psA bufs=2); the
     final group splits normalize+proj into col pieces [0:256]/[256:512]
     so most of it overlaps the last waves and ships early.
Prologue: weights bundled into 2 DMAs (wqk on sync, wrest=wv+wo+tri+id on
sync) + split xg0 on scalar - each ring pays a serial ~2-3us completion
receipt per DMA, so fewer/bundled prologue DMAs start compute ~7us sooner.
"""
import os
import sys

sys.path.insert(0, "/opt/trn_rl_repo")

from contextlib import ExitStack

import numpy as np

import concourse.bass as bass
import concourse.mybir as mybir
import concourse.tile as tile
from concourse import bacc
from concourse.bass_utils import run_bass_kernel_spmd

F32 = mybir.dt.float32
F16 = mybir.dt.float16
MM_DT = F16
MM_NP = np.float16

B, S, H, DH = 2, 2048, 16, 64
D = H * DH            # 1024
T = B * S             # 4096 tokens
NCORES = 8
HPC = H // NCORES     # 2 heads per core
KC = D // 128         # 8 contraction chunks
NG = T // 512         # 8 token groups
GPB = NG // B         # 4 groups per batch
NKV = T // 128        # 32 kv chunks of 128 tokens
EXPFN = mybir.ActivationFunctionType.Exp

# DMA-xbar transpose serializes the whole HWDGE ring under Tile's
# hang-workaround (measured ~1.2us each + completion waits) - PE default.
USE_DMAT = os.environ.get("KERNEL_DMAT") == "1"
SP_DT = F32  # matmul outputs must be fp32 in PSUM


def _body(nc, tc, ctx, t_in, t_out, t_out_dbg=None):
    xt, wq, wk, wv, wrest, bqkv = t_in
    po = t_out

    const = ctx.enter_context(tc.tile_pool(name="const", bufs=1))
    big = ctx.enter_context(tc.tile_pool(name="big", bufs=1))
    xtp = ctx.enter_context(tc.tile_pool(name="xtp", bufs=4))
    ep = ctx.enter_context(tc.tile_pool(name="ep", bufs=8))
    rp = ctx.enter_context(tc.tile_pool(name="rp", bufs=4))
    # 8 bufs = one staging tile per group, zero reuse: a reused ostb's WAR
    # wait rides an aliased DMA-lane counter that other po stores advance,
    # which released the next writer early once O-proj chunks were
    # deferred multiple steps (nondeterministic corruption)
    osp = ctx.enter_context(tc.tile_pool(name="osp", bufs=8))
    vsp = ctx.enter_context(tc.tile_pool(name="vsp", bufs=3))

    # PSUM budget (8 banks):
    #   sp f32 2 banks x2 + ct0/ct1 1 bank x1 each + accop 1 bank x2 = 8
    ct_bufs = 1
    psS = ctx.enter_context(tc.tile_pool(name="psS", bufs=2, space="PSUM"))
    psC = ctx.enter_context(tc.tile_pool(name="psC", bufs=ct_bufs, space="PSUM"))
    psA = ctx.enter_context(tc.tile_pool(name="psA", bufs=2, space="PSUM"))

    # ---- constants / weights in SBUF (bundled: one DMA per ring pays one
    # serial completion receipt, vs ~2-3us each for 7 separate prologue DMAs)
    wq_s = const.tile([128, KC, 128], MM_DT, tag="wq")
    wk_s = const.tile([128, KC, 128], MM_DT, tag="wk")
    wv_s = const.tile([128, KC, 128], MM_DT, tag="wv")
    wrest_s = const.tile([128, D + 256], MM_DT, tag="wrest")
    wo_s = wrest_s[:, 0:D]
    tri_s = wrest_s[:, D:D + 128]
    id_s = wrest_s[:, D + 128:D + 256]
    bqkv_s = const.tile([128, 3], F32, tag="bqkv")
    bq_s, bk_s, bv_s = (bqkv_s[:, 0:1], bqkv_s[:, 1:2], bqkv_s[:, 2:3])
    qT = big.tile([128, T], MM_DT, tag="qT")
    kT = big.tile([128, T], MM_DT, tag="kT")
    vT = big.tile([128, T], MM_DT, tag="vT")
    ctxT = big.tile([128, T], MM_DT, tag="ctxT")
    vaug = big.tile([128, NKV, HPC, 128], MM_DT, tag="vaug")

    woc = wo_s.rearrange("p (c n) -> p c n", c=KC)

    if t_out_dbg is not None:
        dbgp = ctx.enter_context(tc.tile_pool(name="dbgp", bufs=1))
        dbg_craw_s = dbgp.tile([128, T], F32, tag="craw")
        dbg_den_s = dbgp.tile([128, T], F32, tag="den")

    # groups alternate batches so attention work unlocks early and evenly
    steps = [(b, j) for j in range(GPB) for b in range(B)]

    xgs = {}

    def emit_xg_load(idx, engine=None, split=False):
        # prefetch the hidden-state slice for step idx (gpsimd/SWDGE ring so
        # it is not queued behind the po stores on the sync ring); split=True
        # halves the transfer so the first chain can start on the first half
        gg_l = steps[idx][0] * GPB + steps[idx][1]
        xg = xtp.tile([128, KC, 512], MM_DT, tag="xt")
        eng = engine or nc.gpsimd
        if split:
            # 6/2 split: the q chain consumes chunks in order at ~2 chunks
            # per 432ns, so the first piece must cover it until the second
            # piece's ring receipt clears
            eng.dma_start(xg[:, 0:6, :], xt[gg_l][:, 0:6, :])
            eng.dma_start(xg[:, 6:, :], xt[gg_l][:, 6:, :])
        else:
            eng.dma_start(xg[:], xt[gg_l])
        xgs[idx] = xg

    def qkv_quanta(idx):
        """Per-step QKV work broken into small PE quanta so it can be
        interleaved into the previous step's attention waves."""
        (b_q, j_q) = steps[idx]
        gg_q = b_q * GPB + j_q
        cols_q = slice(gg_q * 512, (gg_q + 1) * 512)
        thunks = []
        state = {}
        for (w_s, b_s, dst) in ((wq_s, bq_s, qT), (wk_s, bk_s, kT),
                                (wv_s, bv_s, vT)):
            def chain(k0, w_s=w_s, b_s=b_s, dst=dst):
                if k0 == 0:
                    state['acc'] = psA.tile([128, 512], F32, tag="accop",
                                            name="acc")
                acc = state['acc']
                for k in (k0, k0 + 1):
                    nc.tensor.matmul(acc[:], w_s[:, k, :], xgs[idx][:, k, :],
                                     start=(k == 0), stop=(k == KC - 1))
                if k0 == KC - 2:
                    if idx == 0 and dst is qT:
                        # step 0 races the bqkv prologue DMA: evacuate as a
                        # plain copy (frees the accop buffer for the v
                        # chain immediately) and add the bias in place on
                        # SBUF once bqkv lands - scores need qT ~2us later
                        nc.vector.tensor_copy(dst[:, cols_q], acc[:])
                        nc.vector.tensor_scalar_add(dst[:, cols_q],
                                                    dst[:, cols_q], b_s[:])
                    else:
                        nc.vector.tensor_scalar_add(dst[:, cols_q], acc[:],
                                                    b_s[:])
            for k0 in range(0, KC, 2):
                thunks.append(lambda k0=k0, chain=chain: chain(k0))
        for i in range(4):
            def trans(i=i):
                cg = gg_q * 4 + i
                vslice = vT[:, cg * 128:(cg + 1) * 128]
                tp = psA.tile([128, 128], MM_DT, tag="accop")
                nc.tensor.transpose(tp[:], vslice, id_s[:])
                nc.vector.tensor_copy(
                    vaug[:, cg, :, 0:64],
                    tp[:].rearrange("p (h d) -> p h d", h=HPC))
            thunks.append(trans)
        return thunks

    # PE warmup: dummy matmuls on a memset tile so the HAM clock-gate opens
    # (~3.4us of activity) before the first real QKV chain arrives - no
    # data deps, so these start right after the kernel preamble
    warm = const.tile([128, 512], MM_DT, tag="warm")
    nc.vector.memset(warm[:], 0.0)
    # 12 matmuls ~= 5us at mid-clock: opens the HAM clock-gate AND bridges
    # the idle window until wq + xg0 land (~1.6us past an 8-matmul warmup)
    wacc = psA.tile([128, 512], F32, tag="accop", name="wacc")
    for k in range(12):
        nc.tensor.matmul(wacc[:], warm[:, 0:128], warm[:],
                         start=(k == 0), stop=(k == 11))

    # prologue: each ring pays a serial ~2-3us completion receipt per DMA.
    # need-by times: wq ~11 (q chain), wk ~12.9 (k chain), bqkv ~14 (the q
    # evac frees the accop buffer the v chain allocates), wv ~14.4,
    # tri/id ~16-19. bqkv is tiny so it leads sync without delaying wq
    # much; wv leads the gpsimd/SWDGE ring whose first item lands ~14us.
    nc.sync.dma_start(wq_s[:], wq[:])
    nc.sync.dma_start(wk_s[:], wk[:])
    nc.sync.dma_start(bqkv_s[:], bqkv[:])
    emit_xg_load(0, nc.scalar, split=True)
    nc.gpsimd.dma_start(wv_s[:], wv[:])
    nc.gpsimd.dma_start(wrest_s[:], wrest[:])
    emit_xg_load(1, nc.sync)
    # ones columns 64:128 of V_aug (softmax denominator, replicated over the
    # 64 PSUM partitions 64:127 so normalize needs no partition broadcast).
    # memset, not DMA: a DMA's completion lands on a shared DMA-lane counter
    # that later po writes keep advancing, so every AV matmul would pick up
    # false waits on po-write completions.
    va_ones = vaug[:].rearrange("p c h x -> p (c h) x")[:, :, 64:128]
    nc.gpsimd.memset(va_ones, 1.0)
    emit_xg_load(2)
    for t in qkv_quanta(0):
        t()
    pending_oproj = []

    for idx, (b, j) in enumerate(steps):
        gg = b * GPB + j
        cols = slice(gg * 512, (gg + 1) * 512)
        n_waves = 4 * j + 4
        last_step = (idx == len(steps) - 1)

        # work to interleave into this step's attention waves: deferred
        # output projections, next step's QKV, and an xg prefetch
        from collections import deque as _dq
        quanta = _dq()
        if idx + 3 < len(steps):
            quanta.append(lambda i=idx + 3: emit_xg_load(i))
        # interleave deferred O-proj chunks between QKV thunks: each O-proj
        # matmul WAR-waits on the evac (DVE, ~690ns) of the chunk two back
        # (psA bufs=2), so consecutive O-proj quanta must not be adjacent
        # or the PE stalls on the evac. Take at most one O-proj chunk per
        # wave: early (few-wave) steps would otherwise cluster them, and
        # capping here shifts the backlog into the late attention-heavy
        # steps whose waves run dry of QKV fill.
        nxt = qkv_quanta(idx + 1) if idx + 1 < len(steps) else []
        # from mid-kernel on, hold back ~8 chunks so the final step (which
        # has no next-QKV fill) still has PE work to cover its exposed
        # normalize chains
        reserve = 8 if idx >= 4 else 0
        take = (len(pending_oproj) if last_step else
                min(len(pending_oproj), n_waves,
                    max(0, len(pending_oproj) - reserve)))
        a, bq_ = pending_oproj[:take], list(nxt)
        pending_oproj = pending_oproj[take:]
        ratio = max(1, -(-len(bq_) // max(1, len(a))))
        while a or bq_:
            for _ in range(ratio):
                if bq_:
                    quanta.append(bq_.pop(0))
            if a:
                quanta.append(a.pop(0))
        waves_left = [2 * n_waves]

        def pop_quanta():
            # even draw across remaining half-waves (one draw after exp,
            # one after av)
            k = -(-len(quanta) // max(1, waves_left[0]))
            for _ in range(k):
                if quanta:
                    quanta.popleft()()
            waves_left[0] -= 1

        # ---- attention macro-block (b, j): q cols [gg*512,(gg+1)*512).
        # One wave per kv chunk; N=512 matmuls fully hide the 128-row
        # LDWEIGHTS that N=256 AV matmuls paid ~48ns each for, and the
        # diagonal chunks only stream their valid q range.
        qc0 = b * S + j * 512
        qcols = slice(qc0, qc0 + 512)
        nch = 4 * j + 4                # kv chunks of 128 for this block
        ct0 = psC.tile([128, 512], F32, tag="ct0")
        ct1 = psC.tile([128, 512], F32, tag="ct1")
        cts = [ct0, ct1]

        def av_wave(jj, e, qlo):
            first = (jj == 0)
            last = (jj == nch - 1)
            for h in range(HPC):
                lhsT = vaug[:, b * (S // 128) + jj, h, :]
                nc.tensor.matmul(cts[h][:, qlo:512], lhsT,
                                 e[:, h, qlo:512], start=first, stop=last)

        if last_step:
            # final group: normalize + O-proj split into two column pieces.
            # cols [0:256] of ct are final right after av(nch-3) (the d=1
            # diagonal chunk), so that piece overlaps the last waves; only
            # the [256:512] chain remains exposed after the final av.
            ostF = osp.tile([128, KC, 512], MM_DT, tag="ost", name="ostF")

            def final_piece(c0, c1, gg=gg):
                w = c1 - c0
                for h in range(HPC):
                    dn = rp.tile([64, w], F32, tag=f"dF{c0}")
                    nc.vector.tensor_copy(dn[:], cts[h][64:128, c0:c1])
                    rr = rp.tile([64, w], F32, tag=f"rF{c0}")
                    nc.vector.reciprocal_approx_fast(rr[:], dn[:])
                    nc.vector.tensor_mul(
                        ctxT[h * 64:(h + 1) * 64, qc0 + c0:qc0 + c1],
                        cts[h][0:64, c0:c1], rr[:])
                for c in range(KC):
                    op = psA.tile([128, w], SP_DT, tag="accop", name="opf")
                    nc.tensor.matmul(op[:], woc[:, c, :],
                                     ctxT[:, qc0 + c0:qc0 + c1],
                                     start=True, stop=True)
                    nc.vector.tensor_copy(ostF[:, c, c0:c1], op[:])
                    # ship each piece as soon as its 4-chunk group is done:
                    # piece 1 (cols 0:256) leaves during the last waves, so
                    # only ~512KB of piece 2 remains for the drain to wait on
                    if c % 4 == 3:
                        eng = nc.sync if c == 3 else nc.scalar
                        eng.dma_start(po[gg][:, c - 3:c + 1, c0:c1],
                                      ostF[:, c - 3:c + 1, c0:c1])

        # software pipeline: emit scores(jj)+exp(jj) before AV(jj-1), so
        # the PE stream is S0 S1 A0 S2 A1 ... and exp overlaps AV
        prev = None
        for jj in range(nch):
            d = jj - 4 * j             # diagonal index (>=0 on diagonal)
            qlo = max(0, d * 128)      # q cols below qlo are fully masked
            kcols = slice(b * S + jj * 128, b * S + jj * 128 + 128)
            sp = psS.tile([128, HPC, 512], SP_DT, tag="sp")
            for h in range(HPC):
                nc.tensor.matmul(
                    sp[:, h, qlo:512],
                    kT[h * 64:(h + 1) * 64, kcols],
                    qT[h * 64:(h + 1) * 64, qc0 + qlo:qc0 + 512],
                    start=True, stop=True)
            e = ep.tile([128, HPC, 512], MM_DT, tag="e")
            nc.scalar.activation(e[:, :, qlo:512], sp[:, :, qlo:512],
                                 EXPFN, scale=0.125)
            if d >= 0:                 # mask the diagonal 128x128 triangle
                for h in range(HPC):
                    nc.gpsimd.tensor_mul(e[:, h, qlo:qlo + 128],
                                         e[:, h, qlo:qlo + 128], tri_s)
            pop_quanta()
            if prev is not None:
                av_wave(*prev)
                if last_step and prev[0] == nch - 3:
                    final_piece(0, 256)
            pop_quanta()
            prev = (jj, e, qlo)
        av_wave(*prev)
        if last_step:
            final_piece(256, 512)
        else:
            # normalize: ctxT[:, q] = ct[0:64] * recip(ct[64:128]) - the
            # denominator is already replicated across partitions 64:127
            for h in range(HPC):
                # tensor_copy is the only DVE op that honors a partition
                # shift; custom-DVE (recip) and TT reads do not, so copy the
                # replicated denominators down to partitions 0:63 first
                dn = rp.tile([64, 512], F32, tag="d")
                nc.vector.tensor_copy(dn[:], cts[h][64:128, :])
                r = rp.tile([64, 512], F32, tag="r")
                nc.vector.reciprocal_approx_fast(r[:], dn[:])
                nc.vector.tensor_mul(ctxT[h * 64:(h + 1) * 64, qcols],
                                     cts[h][0:64, :], r[:])
                if t_out_dbg is not None:
                    nc.vector.tensor_copy(
                        dbg_craw_s[h * 64:(h + 1) * 64, qcols],
                        cts[h][0:64, :])
                    nc.vector.tensor_copy(
                        dbg_den_s[h * 64:(h + 1) * 64, qcols],
                        cts[h][64:128, :])

        while quanta:          # leftover next-step QKV work
            quanta.popleft()()

        if not last_step:
            # ---- output projection for the completed 512-token group,
            # deferred into the next step's waves (the last step's is
            # handled by final_piece above).
            ostb = osp.tile([128, KC, 512], MM_DT, tag="ost", name="ostb")

            def proj_chunk(c, cols=cols, ostb=ostb, gg=gg):
                op = psA.tile([128, 512], SP_DT, tag="accop", name="op")
                nc.tensor.matmul(op[:], woc[:, c, :], ctxT[:, cols],
                                 start=True, stop=True)
                nc.vector.tensor_copy(ostb[:, c, :], op[:])
                if c == KC - 1:
                    nc.sync.dma_start(po[gg], ostb[:])

            # bind proj_chunk now: it is redefined each step, and these
            # thunks may run 2+ steps later
            pending_oproj.extend(
                lambda c=c, f=proj_chunk: f(c) for c in range(KC))

    if t_out_dbg is not None:
        dq, dk, dc, dv, dcr, dde = t_out_dbg
        st = ctx.enter_context(tc.tile_pool(name="dbg", bufs=1))
        for src, dst in ((qT, dq), (kT, dk), (ctxT, dc)):
            tmp = st.tile([128, T], F32, tag="dbgt")
            nc.vector.tensor_copy(tmp[:], src[:])
            nc.sync.dma_start(dst[:], tmp[:])
        nc.sync.dma_start(dcr[:], dbg_craw_s[:])
        nc.sync.dma_start(dde[:], dbg_den_s[:])
        tmpv = st.tile([128, NKV * HPC * 128], F32, tag="dbgt")
        nc.vector.tensor_copy(
            tmpv[:], vaug[:].rearrange("p c h x -> p (c h x)"))
        nc.sync.dma_start(dv[:], tmpv[:])


_NC = None


def _build():
    global _NC
    if _NC is not None:
        return _NC
    nc = bacc.Bacc("TRN2", target_bir_lowering=False, debug=False,
                   num_devices=NCORES)
    t_in = [
        nc.dram_tensor("xt", [NG, 128, KC, 512], MM_DT, kind="ExternalInput").ap(),
        nc.dram_tensor("wq", [128, KC, 128], MM_DT, kind="ExternalInput").ap(),
        nc.dram_tensor("wk", [128, KC, 128], MM_DT, kind="ExternalInput").ap(),
        nc.dram_tensor("wv", [128, KC, 128], MM_DT, kind="ExternalInput").ap(),
        nc.dram_tensor("wrest", [128, D + 256], MM_DT,
                       kind="ExternalInput").ap(),
        nc.dram_tensor("bqkv", [128, 3], F32, kind="ExternalInput").ap(),
    ]
    # group-major, partition-major layout: each group store is one DMA with
    # 8KB contiguous per partition (vs 1KB segments in a [KC,128,T] layout)
    po = nc.dram_tensor("po", [NG, 128, KC, 512], MM_DT,
                        kind="ExternalOutput").ap()
    t_out_dbg = None
    if os.environ.get("KERNEL_DEBUG_TAPS") == "1":
        t_out_dbg = [
            nc.dram_tensor("dbg_qT", [128, T], F32, kind="ExternalOutput").ap(),
            nc.dram_tensor("dbg_kT", [128, T], F32, kind="ExternalOutput").ap(),
            nc.dram_tensor("dbg_ctxT", [128, T], F32, kind="ExternalOutput").ap(),
            nc.dram_tensor("dbg_vaug", [128, NKV * HPC * 128], F32,
                           kind="ExternalOutput").ap(),
            nc.dram_tensor("dbg_craw", [128, T], F32, kind="ExternalOutput").ap(),
            nc.dram_tensor("dbg_den", [128, T], F32, kind="ExternalOutput").ap(),
        ]
    with tile.TileContext(nc) as tc, ExitStack() as ctx:
        _body(nc, tc, ctx, t_in, po, t_out_dbg)
    nc.compile()
    _NC = nc
    return nc


def _in_maps(hidden_states, Wq, bq, Wk, bk, Wv, bv, Wo, bo):
    hid = np.asarray(hidden_states, dtype=np.float32).reshape(T, D)
    hidT = hid.T.astype(MM_NP)                       # [D, T]
    xt = np.ascontiguousarray(
        hidT.reshape(KC, 128, NG, 512).transpose(2, 1, 0, 3))
    tri = np.triu(np.ones((128, 128), MM_NP))
    eye = np.eye(128, dtype=MM_NP)
    common = {"xt": xt}
    maps = []
    for c in range(NCORES):
        cs = slice(c * 128, (c + 1) * 128)
        wslc = lambda W: np.asarray(W)[:, cs].astype(MM_NP).reshape(
            KC, 128, 128).transpose(1, 0, 2)
        wo_c = np.asarray(Wo)[cs, :].astype(MM_NP)
        maps.append(dict(
            common,
            wq=np.ascontiguousarray(wslc(Wq)),
            wk=np.ascontiguousarray(wslc(Wk)),
            wv=np.ascontiguousarray(wslc(Wv)),
            wrest=np.ascontiguousarray(np.concatenate(
                [wo_c, tri, eye], axis=1)),
            bqkv=np.ascontiguousarray(np.stack(
                [np.asarray(bq)[cs], np.asarray(bk)[cs],
                 np.asarray(bv)[cs]], axis=1).astype(np.float32)),
        ))
    return maps


def kernel(hidden_states, Wq, bq, Wk, bk, Wv, bv, Wo, bo):
    nc = _build()
    maps = _in_maps(hidden_states, Wq, bq, Wk, bk, Wv, bv, Wo, bo)
    res = run_bass_kernel_spmd(nc, maps, list(range(NCORES))).results
    acc = np.zeros((NG, 128, KC, 512), np.float32)
    for r in res:
        acc += r["po"].astype(np.float32)
    # [NG,128,KC,512] -> [KC*128, NG*512] = [D, T]
    outT = acc.transpose(2, 1, 0, 3).reshape(D, T)
    out = outT.T + np.asarray(bo, dtype=np.float32)[None, :]
    return out.reshape(B, S, D).astype(np.float32)

